# revision 1
# baseline (speedup 1.0000x reference)
"""ChannelAttention kernel for Trainium2 (8 NeuronCores, batch-parallel).

Reference computation per batch element b (C=64, N=H*W=65536):
    X1 = x[b] viewed [C, N]          (proj_query)
    X2 = x[b] viewed [N, C]          (proj_key -- a reshape, NOT a transpose)
    S  = X1 @ X2                     [C, C]
    P  = softmax(S, axis=-1)
    out[b] = (P @ X1) + X1  =  (P + I) @ X1

Sharding: data-parallel over batch. B=16 -> 2 batches per core on 8 cores.

Per-core dataflow (per batch):
  - x[b] resident in SBUF as 16 column-strips [128, 2048] f32: partition c
    holds X1[c, colhalf0-window], partition 64+c holds X1[c, colhalf1-window].
  - mm1 lhsT tiles: PE-transpose of strip slices [128,128] -> X1^T tiles for
    two n-windows at once (cols 0:64 = window u, cols 64:128 = window u+256).
  - mm1 rhs tiles: X2 contraction-major tiles streamed from HBM with a
    strided access pattern ([128, 32, 64] chunks, 1MB contiguous reads).
  - S accumulates over 512 matmuls in one PSUM tile [64, 64].
  - softmax: DVE row-max (negated) -> ACT exp with fused row-sum ->
    DVE reciprocal -> fused (E * 1/sum) + I.
  - (P+I)^T via PE transpose; replicated to partitions 64:128 via tiny
    SBUF->SBUF DMA so both column-halves of mm2 have aligned operands.
  - mm2: 128 matmuls [64p x 64] @ [64p x 512] -> PSUM -> copy (DVE/ACT
    alternating) into [64, 2048] staging -> 512KB stores to HBM.
"""

import numpy as np

_CACHE = {}

B_FULL = 16
C = 64
N = 65536          # H*W = 256*256
NB = 2             # batches per core
NCORES = 8
NWIN = 256         # 128-col windows per column-half (32768 / 128)
NSTRIP = 16        # strips per batch; strip = [128, 2048]
STRIPW = 2048
VCH = 32           # X2 tiles per V chunk (1 MB per chunk)
NCHUNK = 8         # V chunk pairs per batch (8 * 32 = 256 windows)


def _build(loop_reps=1):
    from contextlib import nullcontext

    import concourse.bacc as bacc
    import concourse.mybir as mybir
    import concourse.tile as tile
    from concourse.masks import make_identity

    f32 = mybir.dt.float32
    Alu = mybir.AluOpType
    Act = mybir.ActivationFunctionType

    nc = bacc.Bacc("TRN2", debug=False)
    xb = nc.dram_tensor("xb", [NB, C * N], f32, kind="ExternalInput").ap()
    ob = nc.dram_tensor("ob", [NB, C * N], f32, kind="ExternalOutput").ap()

    with tile.TileContext(nc) as tc:
        with (
            tc.tile_pool(name="consts", bufs=1) as consts,
            tc.tile_pool(name="H", bufs=NSTRIP) as hpool,
            tc.tile_pool(name="V", bufs=3) as vpool,
            tc.tile_pool(name="TOs", bufs=6) as topool,
            tc.tile_pool(name="stage", bufs=2) as stpool,
            tc.tile_pool(name="soft", bufs=2) as softpool,
            tc.tile_pool(name="psT", bufs=3, space="PSUM") as psT,
            tc.tile_pool(name="psS", bufs=1, space="PSUM") as psS,
            tc.tile_pool(name="psO", bufs=2, space="PSUM") as psO,
            tc.tile_pool(name="psP", bufs=1, space="PSUM") as psP,
        ):
            ident = consts.tile([128, 128], f32)
            make_identity(nc, ident[:])

            loop_cm = (
                tc.For_i(0, loop_reps, 1) if loop_reps > 1 else nullcontext()
            )
            with loop_cm:
              for b in range(NB):
                x1 = xb[b].rearrange("(c n) -> c n", c=C)      # [64, 65536]
                o1 = ob[b].rearrange("(c n) -> c n", c=C)

                # ---- load phase: interleave H strips and V chunks ----
                # Each strip is loaded by two 64-partition DMAs placed on the
                # two HWDGE rings (sync/scalar): partitions 0-63 hit the even
                # SBUF AXI ports and 64-127 the odd ones, so the concurrent
                # pair covers all 16 ports.
                strips = []
                vtiles = []
                for j in range(NCHUNK):
                    for k in (2 * j, 2 * j + 1):
                        st = hpool.tile([128, STRIPW], f32, tag="H")
                        nc.sync.dma_start(
                            st[0:64, :], x1[:, k * STRIPW:(k + 1) * STRIPW]
                        )
                        nc.scalar.dma_start(
                            st[64:128, :],
                            x1[:, 32768 + k * STRIPW: 32768 + (k + 1) * STRIPW],
                        )
                        strips.append(st)
                    # V chunk pair: tiles t in [32j, 32j+32) and [256+32j, ...)
                    # interleaved as [128, tl, half, c] so window u's matmul
                    # can take rhs = [U_u | U_{u+256}] as one [128, 128] slice.
                    vt = vpool.tile([128, VCH, 2, C], f32, tag="V")
                    for half in range(2):
                        t0 = 256 * half + VCH * j
                        src = xb[b][t0 * 8192:(t0 + VCH) * 8192].rearrange(
                            "(t p c) -> p t c", p=128, c=C
                        )
                        nc.sync.dma_start(vt[:, :, half, :], src)
                    vtiles.append(vt)

                # ---- mm1: S = X1 @ X2, accumulated over 512 tiles ----
                # One N=128 matmul per window: lhsT = [T_u | T_{u+256}]
                # (from one transpose), rhs = [U_u | U_{u+256}].  The two
                # diagonal 64x64 blocks of the [128, 128] accumulator hold
                # the real contributions; off-diagonal blocks are ignored.
                # PE stream is software-pipelined: transposes for pair p+SKEW
                # are emitted before the matmuls of pair p, so the PE never
                # waits on the PSUM->SBUF copy chain.
                SKEW = 2
                s_ps = psS.tile([128, 128], f32, tag="S")
                npairs = NWIN // 2               # 128 window pairs
                to_sbs = {}

                def emit_transpose(tp):
                    u0 = 2 * tp
                    to_ps = psT.tile([128, 2, 128], f32, tag="TO")
                    for q in range(2):
                        u = u0 + q
                        st = strips[u // 16]
                        ti = st[:, (u % 16) * 128:(u % 16) * 128 + 128]
                        nc.tensor.transpose(to_ps[:, q, :], ti, ident[:])
                    to_sb = topool.tile([128, 2, 128], f32, tag="TOs")
                    if tp % 2 == 0:
                        nc.scalar.copy(to_sb[:], to_ps[:])
                    else:
                        nc.vector.tensor_copy(to_sb[:], to_ps[:])
                    to_sbs[tp] = to_sb

                for tp in range(SKEW):
                    emit_transpose(tp)
                for tp in range(npairs):
                    if tp + SKEW < npairs:
                        emit_transpose(tp + SKEW)
                    to_sb = to_sbs.pop(tp)
                    for q in range(2):
                        u = 2 * tp + q
                        j, tl = u // VCH, u % VCH
                        nc.tensor.matmul(
                            s_ps[:], to_sb[:, q, :],
                            vtiles[j][:, tl, :, :],
                            start=(u == 0), stop=(u == NWIN - 1),
                        )

                # ---- S = UL + LR (diagonal blocks of the accumulator) ----
                s_sb = softpool.tile([128, 128], f32, tag="Ssb")
                nc.vector.tensor_copy(s_sb[:], s_ps[:])
                s_fix = softpool.tile([64, 64], f32, tag="Sfix")
                nc.sync.dma_start(s_fix[:], s_sb[64:128, 64:128])
                s2_sb = softpool.tile([64, 64], f32, tag="S2")
                nc.vector.tensor_add(s2_sb[:], s_sb[0:64, 0:64], s_fix[:])

                # ---- softmax + (P + I), transposed ----
                nmx = softpool.tile([64, 1], f32, tag="nmx")
                nc.vector.tensor_reduce(
                    nmx[:], s2_sb[:], axis=mybir.AxisListType.X, op=Alu.max,
                    negate=True,
                )
                esum = softpool.tile([64, 1], f32, tag="esum")
                e_sb = softpool.tile([64, 64], f32, tag="E")
                nc.scalar.activation(
                    e_sb[:], s2_sb[:], Act.Exp, bias=nmx[:, 0:1], scale=1.0,
                    accum_out=esum[:],
                )
                rcp = softpool.tile([64, 1], f32, tag="rcp")
                nc.vector.reciprocal(rcp[:], esum[:])
                pi_sb = softpool.tile([64, 64], f32, tag="PI")
                # PI = (E * 1/sum) + I
                nc.vector.scalar_tensor_tensor(
                    pi_sb[:], e_sb[:], rcp[:, 0:1], ident[0:64, 0:64],
                    Alu.mult, Alu.add,
                )
                pit_ps = psP.tile([64, 64], f32, tag="PIT")
                nc.tensor.transpose(pit_ps[:], pi_sb[:], ident[0:64, 0:64])
                pit = softpool.tile([128, 64], f32, tag="PITb")
                nc.vector.tensor_copy(pit[0:64, :], pit_ps[:])
                nc.sync.dma_start(pit[64:128, :], pit[0:64, :])

                # ---- mm2: out = (P+I) @ X1, 128 windows of 512 cols ----
                # Output windows packed two-deep across PSUM/SBUF partition
                # halves (tile_position col groups) so stores run at full
                # 128-partition port width and mm2 matmuls pair up on the
                # two array column halves.
                for half in range(2):
                    lhs = pit[64 * half:64 * half + 64, :]
                    for g in range(8):            # groups of 8 windows (4096)
                        stg = stpool.tile([128, 4, 512], f32, tag="stage")
                        for hb in range(2):
                            for wi in range(4):
                                w = g * 8 + hb * 4 + wi
                                st = strips[w // 4]
                                rhs = st[64 * half:64 * half + 64,
                                         (w % 4) * 512:(w % 4) * 512 + 512]
                                o_ps = psO.tile([128, 512], f32, tag="O")
                                nc.tensor.matmul(
                                    o_ps[64 * hb:64 * hb + 64, :], lhs, rhs,
                                    start=True, stop=True,
                                )
                                if w % 2 == 0:
                                    nc.vector.tensor_copy(
                                        stg[64 * hb:64 * hb + 64, wi, :],
                                        o_ps[64 * hb:64 * hb + 64, :],
                                    )
                                else:
                                    nc.scalar.copy(
                                        stg[64 * hb:64 * hb + 64, wi, :],
                                        o_ps[64 * hb:64 * hb + 64, :],
                                    )
                        off = 32768 * half + g * 4096
                        nc.scalar.dma_start(
                            o1[:, off:off + 2048],
                            stg[0:64].rearrange("p a b -> p (a b)"),
                        )
                        nc.sync.dma_start(
                            o1[:, off + 2048:off + 4096],
                            stg[64:128].rearrange("p a b -> p (a b)"),
                        )

    nc.compile()
    return nc


def kernel(x: np.ndarray) -> np.ndarray:
    from concourse.bass_utils import run_bass_kernel_spmd

    if "nc" not in _CACHE:
        _CACHE["nc"] = _build()
    nc = _CACHE["nc"]

    x = np.ascontiguousarray(x, dtype=np.float32)
    B, Cc, H, W = x.shape
    xflat = x.reshape(B, Cc * H * W)
    in_maps = [
        {"xb": xflat[NB * i:NB * (i + 1)]} for i in range(NCORES)
    ]
    res = run_bass_kernel_spmd(nc, in_maps, core_ids=list(range(NCORES)))
    out = np.empty_like(xflat)
    for i in range(NCORES):
        out[NB * i:NB * (i + 1)] = res.results[i]["ob"]
    return out.reshape(B, Cc, H, W)



# revision 2
# speedup vs baseline: 9.2853x; 9.2853x over previous
"""ChannelAttention kernel v2 for Trainium2 (8 NeuronCores, batch-parallel).

Per batch element b (C=64, N=H*W=65536):
    X1 = x[b] viewed [C, N]; X2 = x[b] viewed [N, C] (reshape, NOT transpose)
    S  = X1 @ X2;  P = softmax(S);  out[b] = (P + I) @ X1

Key design points vs v1:
  - mm1 in fp16 with hi/lo error compensation: X ~ A + a, X2 ~ B + b2
    (A,B = fp16 roundings; a,b2 = fp16 residuals). S accumulates
    A@B + A@b2 + a@B in one fp32 PSUM group (1536 matmuls of
    [128,64]@[128,64], 1 cyc/row instead of fp32's 4).
    Validated numerics: rel err ~1.1e-3 vs fp32 reference.
  - One fp32 PE-transpose pass serves both A^T and a^T (extraction
    copies do the fp16 cast / residual subtract from PSUM).
  - V (X2-layout) loads use [128,16,64] tiles: 4KB contiguous per
    partition line instead of 256B chunks.
  - mm2 packs both column-halves via a block-diagonal (P+I)^T lhsT
    [128,128]: one [128,512]-out matmul per 512-col window pair.
  - mm2 outputs staged [128,4,512] in SBUF so stores are 8KB/partition.
"""

import numpy as np

_CACHE = {}

B_FULL = 16
C = 64
N = 65536
NB = 2
NCORES = 8
NCHUNK = 16        # strip/V chunk pairs per batch; chunk = 2048 cols
STRIPW = 2048
RPT = 16           # rows packed per V-tile partition (4KB lines)

MM2_MODE = "f32"   # "f32r" | "f32"  (f32r crashes at runtime on this stack)
MM1_MODE = "f16x3" # "f16x3" (hi/lo compensated fp16) | "f32"


def _build(loop_reps=1, mm2_mode=MM2_MODE, mm1_mode=MM1_MODE):
    three_set = True
    from contextlib import nullcontext

    import concourse.bacc as bacc
    import concourse.mybir as mybir
    import concourse.tile as tile
    from concourse.masks import make_identity

    f32 = mybir.dt.float32
    f32r = mybir.dt.float32r
    f16 = mybir.dt.float16
    Alu = mybir.AluOpType
    Act = mybir.ActivationFunctionType

    nc = bacc.Bacc("TRN2", debug=False)
    xb = nc.dram_tensor("xb", [NB, C * N], f32, kind="ExternalInput").ap()
    ob = nc.dram_tensor("ob", [NB, C * N], f32, kind="ExternalOutput").ap()

    with tile.TileContext(nc) as tc:
        with (
            tc.tile_pool(name="consts", bufs=1) as consts,
            tc.tile_pool(name="H", bufs=NCHUNK) as hpool,
            tc.tile_pool(name="V32", bufs=3) as vpool,
            tc.tile_pool(name="V16", bufs=6) as v16pool,
            tc.tile_pool(name="AT", bufs=8) as atpool,
            tc.tile_pool(name="soft", bufs=2) as softpool,
            tc.tile_pool(name="stage", bufs=2) as stpool,
            tc.tile_pool(name="psT", bufs=3, space="PSUM") as psT,
            tc.tile_pool(name="psS", bufs=1, space="PSUM") as psS,
            tc.tile_pool(name="psP", bufs=1, space="PSUM") as psP,
            tc.tile_pool(name="psO", bufs=3, space="PSUM") as psO,
        ):
            ident = consts.tile([128, 128], f32)
            make_identity(nc, ident[:])

            loop_cm = (
                tc.For_i(0, loop_reps, 1) if loop_reps > 1 else nullcontext()
            )
            with loop_cm:
              for b in range(NB):
                x1 = xb[b].rearrange("(c n) -> c n", c=C)      # [64, 65536]
                o1 = ob[b].rearrange("(c n) -> c n", c=C)

                SKEW = 2
                s_ps = psS.tile([64, 64], f32, tag="S")
                n_sets = 1 if mm1_mode == "f32" else (3 if three_set else 2)
                n_mm1 = NCHUNK * RPT * 2 * n_sets
                mm_idx = 0
                strips = []
                pend = []          # (AT, aT) tiles skewed ahead
                vcur = []          # converted V tiles per chunk

                def emit_chunk_loads(j):
                    st = hpool.tile([128, STRIPW], f32, tag="H")
                    nc.sync.dma_start(
                        st[0:64, :], x1[:, j * STRIPW:(j + 1) * STRIPW]
                    )
                    nc.scalar.dma_start(
                        st[64:128, :],
                        x1[:, 32768 + j * STRIPW:32768 + (j + 1) * STRIPW],
                    )
                    strips.append(st)
                    pair = []
                    for h in range(2):
                        n0 = 32768 * h + 2048 * j
                        v32 = vpool.tile([128, RPT, C], f32, tag="V32")
                        src = xb[b][n0 * C:(n0 + 2048) * C].rearrange(
                            "(p r c) -> p r c", p=128, c=C
                        )
                        (nc.sync if h == 0 else nc.scalar).dma_start(v32[:], src)
                        if mm1_mode == "f32":
                            pair.append((v32, None))
                            continue
                        B16 = v16pool.tile([128, RPT, C], f16, tag="B16")
                        nc.scalar.copy(B16[:], v32[:])
                        b16 = v16pool.tile([128, RPT, C], f16, tag="b16")
                        (nc.vector if h == 0 else nc.gpsimd).tensor_tensor(
                            b16[:], v32[:], B16[:], Alu.subtract
                        )
                        pair.append((B16, b16))
                    vcur.append(pair)

                def emit_transpose(j, r):
                    st3 = strips[j][:].rearrange("p (k r) -> p k r", r=RPT)
                    tp = psT.tile([128, 128], f32, tag="T")
                    nc.tensor.transpose(tp[:], st3[:, :, r], ident[:])
                    if mm1_mode == "f32":
                        AT = atpool.tile([128, 128], f32, tag="AT32")
                        (nc.scalar.copy if r % 2 == 0 else nc.vector.tensor_copy)(
                            AT[:], tp[:])
                        pend.append((AT, None))
                        return
                    AT = atpool.tile([128, 128], f16, tag="AT")
                    aT = atpool.tile([128, 128], f16, tag="aT")
                    (nc.scalar.copy if r % 2 == 0 else nc.vector.tensor_copy)(
                        AT[:], tp[:])
                    nc.vector.tensor_tensor(aT[:], tp[:], AT[:], Alu.subtract)
                    pend.append((AT, aT))

                # ---- load + mm1 pipeline ----
                emit_chunk_loads(0)
                for r in range(SKEW):
                    emit_transpose(0, r)
                for j in range(NCHUNK):
                    if j + 1 < NCHUNK:
                        emit_chunk_loads(j + 1)
                    for r in range(RPT):
                        rs, js = r + SKEW, j
                        if rs >= RPT:
                            rs, js = rs - RPT, j + 1
                        if js < NCHUNK:
                            emit_transpose(js, rs)
                        AT, aT = pend.pop(0)
                        for h in range(2):
                            B16, b16 = vcur[j][h]
                            lA = AT[:, 64 * h:64 * h + 64]
                            if mm1_mode == "f32":
                                sets = [(lA, B16)]
                            else:
                                la = aT[:, 64 * h:64 * h + 64]
                                sets = [(lA, B16), (lA, b16)]
                                if three_set:
                                    sets.append((la, B16))
                            for lhsT, rhs in sets:
                                nc.tensor.matmul(
                                    s_ps[:], lhsT, rhs[:, r, :],
                                    start=(mm_idx == 0),
                                    stop=(mm_idx == n_mm1 - 1),
                                )
                                mm_idx += 1
                    vcur[j] = None

                # ---- softmax + blockdiag (P+I)^T ----
                s_sb = softpool.tile([64, 64], f32, tag="Ssb")
                nc.vector.tensor_copy(s_sb[:], s_ps[:])
                nmx = softpool.tile([64, 1], f32, tag="nmx")
                nc.vector.tensor_reduce(
                    nmx[:], s_sb[:], axis=mybir.AxisListType.X, op=Alu.max,
                    negate=True,
                )
                esum = softpool.tile([64, 1], f32, tag="esum")
                e_sb = softpool.tile([64, 64], f32, tag="E")
                nc.scalar.activation(
                    e_sb[:], s_sb[:], Act.Exp, bias=nmx[:, 0:1], scale=1.0,
                    accum_out=esum[:],
                )
                rcp = softpool.tile([64, 1], f32, tag="rcp")
                nc.vector.reciprocal(rcp[:], esum[:])
                pi_sb = softpool.tile([64, 64], f32, tag="PI")
                nc.vector.scalar_tensor_tensor(
                    pi_sb[:], e_sb[:], rcp[:, 0:1], ident[0:64, 0:64],
                    Alu.mult, Alu.add,
                )
                pit_ps = psP.tile([64, 64], f32, tag="PIT")
                nc.tensor.transpose(pit_ps[:], pi_sb[:], ident[0:64, 0:64])
                pit = softpool.tile([128, 128], f32, tag="PITb")
                nc.vector.tensor_copy(pit[0:64, 0:64], pit_ps[:])
                nc.vector.memset(pit[0:64, 64:128], 0.0)
                nc.vector.memset(pit[64:128, 0:64], 0.0)
                nc.sync.dma_start(pit[64:128, 64:128], pit[0:64, 0:64])

                # ---- mm2 + staged stores ----
                stg = None
                for w in range(64):
                    st = strips[w // 4]
                    rhs = st[:, (w % 4) * 512:(w % 4) * 512 + 512]
                    lhsT = pit[:]
                    if mm2_mode == "f32r":
                        rhs = rhs.bitcast(f32r)
                        lhsT = lhsT.bitcast(f32r)
                    o_ps = psO.tile([128, 512], f32, tag="O")
                    nc.tensor.matmul(o_ps[:], lhsT, rhs, start=True, stop=True)
                    if w % 4 == 0:
                        stg = stpool.tile([128, 4, 512], f32, tag="stage")
                    nc.vector.tensor_copy(stg[:, w % 4, :], o_ps[:])
                    if w % 4 == 3:
                        w0 = (w // 4) * 4
                        nc.sync.dma_start(
                            o1[:, w0 * 512:(w0 + 4) * 512],
                            stg[0:64].rearrange("p a b -> p (a b)"),
                        )
                        nc.scalar.dma_start(
                            o1[:, 32768 + w0 * 512:32768 + (w0 + 4) * 512],
                            stg[64:128].rearrange("p a b -> p (a b)"),
                        )
                strips.clear()

    nc.compile()
    return nc


def kernel(x: np.ndarray) -> np.ndarray:
    from concourse.bass_utils import run_bass_kernel_spmd

    if "nc" not in _CACHE:
        _CACHE["nc"] = _build()
    nc = _CACHE["nc"]

    x = np.ascontiguousarray(x, dtype=np.float32)
    B, Cc, H, W = x.shape
    xflat = x.reshape(B, Cc * H * W)
    in_maps = [
        {"xb": xflat[NB * i:NB * (i + 1)]} for i in range(NCORES)
    ]
    res = run_bass_kernel_spmd(nc, in_maps, core_ids=list(range(NCORES)))
    out = np.empty_like(xflat)
    for i in range(NCORES):
        out[NB * i:NB * (i + 1)] = res.results[i]["ob"]
    return out.reshape(B, Cc, H, W)


# revision 3
# speedup vs baseline: 9.9790x; 1.0747x over previous
"""ChannelAttention kernel v2 for Trainium2 (8 NeuronCores, batch-parallel).

Per batch element b (C=64, N=H*W=65536):
    X1 = x[b] viewed [C, N]; X2 = x[b] viewed [N, C] (reshape, NOT transpose)
    S  = X1 @ X2;  P = softmax(S);  out[b] = (P + I) @ X1

Key design points vs v1:
  - mm1 in fp16 with hi/lo error compensation: X ~ A + a, X2 ~ B + b2
    (A,B = fp16 roundings; a,b2 = fp16 residuals). S accumulates
    A@B + A@b2 + a@B in one fp32 PSUM group (1536 matmuls of
    [128,64]@[128,64], 1 cyc/row instead of fp32's 4).
    Validated numerics: rel err ~1e-4 on hardware vs fp32 reference.
  - One fp32 PE-transpose pass serves both A^T and a^T (extraction
    copies do the fp16 cast / residual subtract from PSUM).
  - V (X2-layout) loads use [128,16,64] tiles: 4KB contiguous per
    partition line instead of 256B chunks (308 vs 217 GB/s measured).
  - mm2 packs both column-halves via a block-diagonal (P+I)^T lhsT
    [128,128]: one [128,512]-out matmul per 512-col window pair.
  - mm2 outputs staged [128,4,512] in SBUF so stores are 8KB/partition.
  - Batch b+1's strip loads are emitted inside batch b's mm2/store loop,
    paired with the strip buffers mm2 frees, so the next batch's load
    stream overlaps the previous batch's store tail instead of queuing
    behind it in ring-FIFO order.
"""

import numpy as np

_CACHE = {}

B_FULL = 16
C = 64
N = 65536
NB = 2
NCORES = 8
NCHUNK = 16        # strip/V chunk pairs per batch; chunk = 2048 cols
STRIPW = 2048
RPT = 16           # rows packed per V-tile partition (4KB lines)

MM2_MODE = "f32"   # "f32r" | "f32"  (f32r crashes at runtime on this stack)
MM1_MODE = "f16x3" # "f16x3" (hi/lo compensated fp16) | "f32"


def _build(loop_reps=1, mm2_mode=MM2_MODE, mm1_mode=MM1_MODE, phase="full"):
    three_set = True
    from contextlib import nullcontext

    import concourse.bacc as bacc
    import concourse.mybir as mybir
    import concourse.tile as tile
    from concourse.masks import make_identity

    f32 = mybir.dt.float32
    f32r = mybir.dt.float32r
    f16 = mybir.dt.float16
    Alu = mybir.AluOpType
    Act = mybir.ActivationFunctionType

    nc = bacc.Bacc("TRN2", debug=False)
    xb = nc.dram_tensor("xb", [NB, C * N], f32, kind="ExternalInput").ap()
    ob = nc.dram_tensor("ob", [NB, C * N], f32, kind="ExternalOutput").ap()

    with tile.TileContext(nc) as tc:
        with (
            tc.tile_pool(name="consts", bufs=1) as consts,
            tc.tile_pool(name="H", bufs=NCHUNK) as hpool,
            tc.tile_pool(name="V32", bufs=4) as vpool,
            tc.tile_pool(name="V16", bufs=6) as v16pool,
            tc.tile_pool(name="AT", bufs=8) as atpool,
            tc.tile_pool(name="soft", bufs=2) as softpool,
            tc.tile_pool(name="stage", bufs=2) as stpool,
            tc.tile_pool(name="psT", bufs=3, space="PSUM") as psT,
            tc.tile_pool(name="psS", bufs=1, space="PSUM") as psS,
            tc.tile_pool(name="psP", bufs=1, space="PSUM") as psP,
            tc.tile_pool(name="psO", bufs=3, space="PSUM") as psO,
        ):
            ident = consts.tile([128, 128], f32)
            make_identity(nc, ident[:])

            loop_cm = (
                tc.For_i(0, loop_reps, 1) if loop_reps > 1 else nullcontext()
            )
            with loop_cm:
              state = [
                  {"strips": [], "vcur": [], "pend": []} for _ in range(NB)
              ]

              def emit_strip_load(b, j):
                  x1 = xb[b].rearrange("(c n) -> c n", c=C)
                  st = hpool.tile([128, STRIPW], f32, tag="H")
                  nc.sync.dma_start(
                      st[0:64, :], x1[:, j * STRIPW:(j + 1) * STRIPW]
                  )
                  nc.scalar.dma_start(
                      st[64:128, :],
                      x1[:, 32768 + j * STRIPW:32768 + (j + 1) * STRIPW],
                  )
                  state[b]["strips"].append(st)

              def emit_v_load(b, j):
                  pair = []
                  for h in range(2):
                      n0 = 32768 * h + 2048 * j
                      v32 = vpool.tile([128, RPT, C], f32, tag="V32")
                      src = xb[b][n0 * C:(n0 + 2048) * C].rearrange(
                          "(p r c) -> p r c", p=128, c=C
                      )
                      (nc.sync if h == 0 else nc.scalar).dma_start(v32[:], src)
                      if mm1_mode == "f32":
                          pair.append((v32, None))
                          continue
                      B16 = v16pool.tile([128, RPT, C], f16, tag="B16")
                      nc.scalar.copy(B16[:], v32[:])
                      b16 = v16pool.tile([128, RPT, C], f16, tag="b16")
                      (nc.vector if h == 0 else nc.gpsimd).tensor_tensor(
                          b16[:], v32[:], B16[:], Alu.subtract
                      )
                      pair.append((B16, b16))
                  state[b]["vcur"].append(pair)

              def emit_transpose(b, j, r):
                  st3 = state[b]["strips"][j][:].rearrange(
                      "p (k r) -> p k r", r=RPT)
                  tp = psT.tile([128, 128], f32, tag="T")
                  nc.tensor.transpose(tp[:], st3[:, :, r], ident[:])
                  if mm1_mode == "f32":
                      AT = atpool.tile([128, 128], f32, tag="AT32")
                      (nc.scalar.copy if r % 2 == 0 else nc.vector.tensor_copy)(
                          AT[:], tp[:])
                      state[b]["pend"].append((AT, None))
                      return
                  AT = atpool.tile([128, 128], f16, tag="AT")
                  aT = atpool.tile([128, 128], f16, tag="aT")
                  (nc.scalar.copy if r % 2 == 0 else nc.vector.tensor_copy)(
                      AT[:], tp[:])
                  nc.vector.tensor_tensor(aT[:], tp[:], AT[:], Alu.subtract)
                  state[b]["pend"].append((AT, aT))

              for b in range(NB):
                s = state[b]
                o1 = ob[b].rearrange("(c n) -> c n", c=C)

                SKEW = 2
                s_ps = psS.tile([64, 64], f32, tag="S")
                n_sets = 1 if mm1_mode == "f32" else (3 if three_set else 2)
                n_mm1 = NCHUNK * RPT * 2 * n_sets
                mm_idx = 0

                # ---- load + mm1 pipeline ----
                if len(s["strips"]) == 0:
                    emit_strip_load(b, 0)
                emit_v_load(b, 0)
                if phase != "nomm1":
                    for r in range(SKEW):
                        emit_transpose(b, 0, r)
                for j in range(NCHUNK):
                    if j + 1 < NCHUNK:
                        if len(s["strips"]) <= j + 1:
                            emit_strip_load(b, j + 1)
                        emit_v_load(b, j + 1)
                    for r in range(RPT):
                        rs, js = r + SKEW, j
                        if rs >= RPT:
                            rs, js = rs - RPT, j + 1
                        if phase == "nomm1":
                            continue
                        if js < NCHUNK:
                            emit_transpose(b, js, rs)
                        AT, aT = s["pend"].pop(0)
                        for h in range(2):
                            B16, b16 = s["vcur"][j][h]
                            lA = AT[:, 64 * h:64 * h + 64]
                            if mm1_mode == "f32":
                                sets = [(lA, B16)]
                            else:
                                la = aT[:, 64 * h:64 * h + 64]
                                sets = [(lA, B16), (lA, b16)]
                                if three_set:
                                    sets.append((la, B16))
                            for lhsT, rhs in sets:
                                nc.tensor.matmul(
                                    s_ps[:], lhsT, rhs[:, r, :],
                                    start=(mm_idx == 0),
                                    stop=(mm_idx == n_mm1 - 1),
                                )
                                mm_idx += 1
                    s["vcur"][j] = None

                if phase == "loads":
                    s["strips"].clear()
                    continue
                # ---- softmax + blockdiag (P+I)^T ----
                if phase == "nomm1":
                    pit = softpool.tile([128, 128], f32, tag="PITb")
                    nc.vector.memset(pit[:], 0.0)
                else:
                  s_sb = softpool.tile([64, 64], f32, tag="Ssb")
                  nc.vector.tensor_copy(s_sb[:], s_ps[:])
                  nmx = softpool.tile([64, 1], f32, tag="nmx")
                  nc.vector.tensor_reduce(
                      nmx[:], s_sb[:], axis=mybir.AxisListType.X, op=Alu.max,
                      negate=True,
                  )
                  esum = softpool.tile([64, 1], f32, tag="esum")
                  e_sb = softpool.tile([64, 64], f32, tag="E")
                  nc.scalar.activation(
                      e_sb[:], s_sb[:], Act.Exp, bias=nmx[:, 0:1], scale=1.0,
                      accum_out=esum[:],
                  )
                  rcp = softpool.tile([64, 1], f32, tag="rcp")
                  nc.vector.reciprocal(rcp[:], esum[:])
                  pi_sb = softpool.tile([64, 64], f32, tag="PI")
                  nc.vector.scalar_tensor_tensor(
                      pi_sb[:], e_sb[:], rcp[:, 0:1], ident[0:64, 0:64],
                      Alu.mult, Alu.add,
                  )
                  pit_ps = psP.tile([64, 64], f32, tag="PIT")
                  nc.tensor.transpose(pit_ps[:], pi_sb[:], ident[0:64, 0:64])
                  pit = softpool.tile([128, 128], f32, tag="PITb")
                  nc.vector.tensor_copy(pit[0:64, 0:64], pit_ps[:])
                  nc.vector.memset(pit[0:64, 64:128], 0.0)
                  nc.vector.memset(pit[64:128, 0:64], 0.0)
                  nc.sync.dma_start(pit[64:128, 64:128], pit[0:64, 0:64])

                if phase == "noout":
                    s["strips"].clear()
                    continue
                # ---- mm2 + staged stores, interleaved with next batch's
                # strip prefetch (strip buffer g frees right after store
                # group g issues) ----
                stg = None
                for w in range(64):
                    st = s["strips"][w // 4]
                    rhs = st[:, (w % 4) * 512:(w % 4) * 512 + 512]
                    lhsT = pit[:]
                    if mm2_mode == "f32r":
                        rhs = rhs.bitcast(f32r)
                        lhsT = lhsT.bitcast(f32r)
                    o_ps = psO.tile([128, 512], f32, tag="O")
                    nc.tensor.matmul(o_ps[:], lhsT, rhs, start=True, stop=True)
                    if w % 4 == 0:
                        stg = stpool.tile([128, 4, 512], f32, tag="stage")
                    (nc.vector.tensor_copy if w % 2 == 0 else nc.scalar.copy)(
                        stg[:, w % 4, :], o_ps[:])
                    if w % 4 == 3:
                        w0 = (w // 4) * 4
                        nc.sync.dma_start(
                            o1[:, w0 * 512:(w0 + 4) * 512],
                            stg[0:64].rearrange("p a b -> p (a b)"),
                        )
                        nc.scalar.dma_start(
                            o1[:, 32768 + w0 * 512:32768 + (w0 + 4) * 512],
                            stg[64:128].rearrange("p a b -> p (a b)"),
                        )
                        if b + 1 < NB:
                            emit_strip_load(b + 1, w // 4)
                s["strips"].clear()

    nc.compile()
    return nc


def kernel(x: np.ndarray) -> np.ndarray:
    from concourse.bass_utils import run_bass_kernel_spmd

    if "nc" not in _CACHE:
        _CACHE["nc"] = _build()
    nc = _CACHE["nc"]

    x = np.ascontiguousarray(x, dtype=np.float32)
    B, Cc, H, W = x.shape
    xflat = x.reshape(B, Cc * H * W)
    in_maps = [
        {"xb": xflat[NB * i:NB * (i + 1)]} for i in range(NCORES)
    ]
    res = run_bass_kernel_spmd(nc, in_maps, core_ids=list(range(NCORES)))
    out = np.empty_like(xflat)
    for i in range(NCORES):
        out[NB * i:NB * (i + 1)] = res.results[i]["ob"]
    return out.reshape(B, Cc, H, W)
